# revision 37
# baseline (speedup 1.0000x reference)
"""Multi-head attention (B=8, N=1024, C=768, H=12) for 8 Trainium2 NeuronCores.

Sharding: data-parallel over the batch dim — core b computes batch element b.
Weights are replicated; no collectives.

v4: everything bf16 (6.4e-3 max rel err vs fp32 reference), and the PE is kept
saturated end-to-end:
  - dummy warmup matmuls run while the first loads land, so the PE's DVFS
    ramp (1.2GHz for ~3us after idle) is spent on junk, not on the V GEMM;
  - pair j+1's qk GEMM is interleaved INTO pair j's attention units (2
    matmuls per unit) instead of running as a block the ACT engine idles
    through — the middle phase is PE-paced at ~20us/pair;
  - V'' is padded to 66 rows/head (ones at 64, zeros at 65) so the PV
    accumulate writes an even partition count;
  - all mid-kernel DMAs issue from the Sync engine's HARDWARE dge queue; the
    GpSimd SOFTWARE queue only takes startup loads (its end-of-kernel drain
    cost scales with lifetime issue count);
  - softmax denominators: DRAM-bounce spread -> [128,4] reciprocal ->
    bounce back, all staged across TWO pair boundaries so no DVE op ever
    waits on an in-flight DMA; the last pair takes a latency-optimized path
    (reciprocal_approx_fast on the [1,512] row, one bounce) so proj isn't
    gated on a 5-hop chain.

Per-core plan (layouts picked so that NO on-device transposes are needed):
  host feeds xT=[C,N] bf16 (x[b].T), wqkp=[6*C,256] bf16 (per-pair [wq|wk]
  column blocks), wv=[C,C] bf16, wpT=[C,C] bf16, bproj=[C] f32.
  1. V GEMM:      V[n, dv]  = xT_chunk.T @ wv            (natural layout)
  2. qT/kT GEMM:  qkT[d, n] = wqk_chunk.T @ xT           (d on partitions)
  3. attention per head pair: S^T = kT_chunk.T @ qT ; expS = exp(SCALE*S^T) ;
     O'[66, nq] += V''_chunk.T @ expS  (row 64 = denom via the ones column)
  4. proj:        y[n, d2] = attnT_chunk.T @ wpT + bproj
"""

import sys

for _p in ("/opt/trn_rl_repo", "/opt/pypackages"):
    if _p not in sys.path:
        sys.path.append(_p)

import numpy as np

import concourse.bass as bass
import concourse.tile as tile
from concourse import bacc, mybir
from concourse.bass_utils import run_bass_kernel_spmd

B, N, C = 8, 1024, 768
H, HD = 12, 64
SCALE = HD**-0.5
NCORES = 8
KC = C // 128  # 6 contraction chunks over C
NT = N // 128  # 8 chunks over sequence (nk / n-tiles)
NQT = N // 512  # 2 moving-dim tiles over the query sequence
PAIRS = H // 2  # 6 head pairs
VW = 128  # V'' row width per head: 64 d + ones@64 + zero pad to full partitions
F32 = mybir.dt.float32
BF16 = mybir.dt.bfloat16
EXP = mybir.ActivationFunctionType.Exp
NWARM = 24  # dummy warmup matmuls (~5us) to hold the PE's DVFS at speed


def _emit(tc, nc, xT, wqkp, wv, wpT, bproj, y, ctx):
    persist = ctx.enter_context(tc.tile_pool(name="persist", bufs=1))
    wqk_pool = ctx.enter_context(tc.tile_pool(name="wqk", bufs=3))
    work = ctx.enter_context(tc.tile_pool(name="work", bufs=3))
    expp = ctx.enter_context(tc.tile_pool(name="expp", bufs=6))
    rdp = ctx.enter_context(tc.tile_pool(name="rdp", bufs=8))
    dram_scr = ctx.enter_context(tc.tile_pool(name="dram_scr", bufs=8, space="DRAM"))
    # 8 PSUM banks: ps_big = 2 x [128,1024] (2 banks each), ps_q = 2 x
    # [128,512] (1 bank each, qk psq halves), ps_o = 2 x [66,512] (1 bank
    # each, O' accumulators).
    ps_big = ctx.enter_context(tc.tile_pool(name="ps_big", bufs=2, space="PSUM"))
    ps_q = ctx.enter_context(tc.tile_pool(name="ps_q", bufs=2, space="PSUM"))
    ps_o = ctx.enter_context(tc.tile_pool(name="ps_o", bufs=2, space="PSUM"))

    # ---- persistent loads ----
    # One dma per [128,*] chunk tile, round-robin across the three DMA-capable
    # issue engines (a 128-row issue costs ~1.6us on the issuing engine, so
    # spreading issues is what bounds time-to-first-matmul). wv/xT ordered
    # kc-major so the V GEMM starts on chunk 0. After startup, gpsimd issues
    # NOTHING (its software dge queue's end-drain scales with issue count).
    eng3 = [nc.sync, nc.scalar, nc.gpsimd]
    nload = 0
    xTs = []
    wvs = []
    for kc in range(KC):
        tv = persist.tile([128, C], BF16, tag=f"wv{kc}", name=f"wv{kc}")
        tx = persist.tile([128, N], BF16, tag=f"xT{kc}", name=f"xT{kc}")
        eng3[nload % 3].dma_start(out=tv, in_=wv[kc * 128 : (kc + 1) * 128, :])
        nload += 1
        eng3[nload % 3].dma_start(out=tx, in_=xT[kc * 128 : (kc + 1) * 128, :])
        nload += 1
        xTs.append(tx)
        wvs.append(tv)
    bpb = persist.tile([128, C], F32, tag="bpb")
    nc.gpsimd.dma_start(
        out=bpb,
        in_=bass.AP(tensor=bproj.tensor, offset=bproj.offset, ap=[[0, 128]] + list(bproj.ap)),
    )

    def load_wqk(j, eng):
        # One dma for the whole pair: wqkp rows j*C:(j+1)*C are the [wq|wk]
        # [C,256] block; 3D AP folds the 6 contraction chunks into columns.
        t = wqk_pool.tile([128, KC * 256], BF16, tag="wqk", name=f"wqk{j}")
        eng.dma_start(
            out=t.rearrange("p (k c) -> p k c", c=256),
            in_=bass.AP(
                tensor=wqkp.tensor,
                offset=wqkp.offset + j * C * 256,
                ap=[[256, 128], [128 * 256, KC], [1, 256]],
            ),
        )
        return t

    def emit_wp_loads():
        wps = []
        for kc in range(KC):
            t = persist.tile([128, C], BF16, tag=f"wp{kc}", name=f"wp{kc}")
            nc.sync.dma_start(out=t, in_=wpT[kc * 128 : (kc + 1) * 128, :])
            wps.append(t)
        return wps

    # ---- phase A: PE warmup on junk data while the loads land ----
    def emit_warmup():
        dummy = work.tile([128, 512], BF16, tag="warm")
        nc.vector.memset(dummy, 0.0)
        for i in range(NWARM):
            psd = ps_o.tile([128, 512], F32, tag="ps_o", name="warm")
            nc.tensor.matmul(psd, dummy[:, 0:128], dummy)

    # ---- phase B: V GEMM (natural layout, head-strided, ones + pad cols) ----
    v2s = []
    for nt in range(NT):
        v2s.append(persist.tile([128, H * VW], BF16, tag=f"v2{nt}", name=f"v2{nt}"))

    def emit_v():
        for nt in range(NT):
            psv = ps_big.tile([128, 1024], F32, tag="ps_big", name="psv")
            for kc in range(KC):
                lhsT = xTs[kc][:, nt * 128 : (nt + 1) * 128]
                nc.tensor.matmul(
                    psv[:, 0:512], lhsT, wvs[kc][:, 0:512], start=(kc == 0), stop=(kc == KC - 1)
                )
                nc.tensor.matmul(
                    psv[:, 512:768], lhsT, wvs[kc][:, 512:768], start=(kc == 0), stop=(kc == KC - 1)
                )
            v2v = v2s[nt].rearrange("p (h e) -> p h e", e=VW)
            # ACT copy: the scalar engine is idle during phase B and drains
            # PSUM faster than the DVE (0.83 vs 1.04 ns/col).
            nc.scalar.copy(
                out=v2v[:, :, 0:HD], in_=psv[:, 0:768].rearrange("p (h e) -> p h e", e=HD)
            )
            nc.vector.memset(v2v[:, :, HD : HD + 1], 1.0)
            nc.vector.memset(v2v[:, :, HD + 1 : VW], 0.0)

    # ---- phases C (qk GEMM) + D (attention) ----
    # qk GEMM for pair j+1 is emitted as a list of closures that emit_attn(j)
    # drains at ~2 matmuls per attention unit, keeping the PE saturated while
    # the ACT engine paces the EXP stream.
    def qk_ops(j, qT, kT, wt):
        # Op order exploits the ps_q 2-slot rotation: group g's psum slot is
        # only reallocated two groups later, so only the first two casts are
        # deadline-critical; casts 3+4 ride at the very end of the pair, far
        # from the O'-drain window, and are still done long before the next
        # pair's S matmuls read qT/kT.
        mms = []
        casts = []
        # k first, then q: attention's first S matmul needs q's cast last.
        for dst, base in ((kT, 128), (qT, 0)):
            for h2 in range(NQT):
                psq = []  # box for the psum tile, allocated by the first op

                def mk_mm(kc, dst=dst, base=base, h2=h2, psq=psq):
                    def op():
                        if kc == 0:
                            psq.append(ps_q.tile([128, 512], F32, tag="ps_q", name="psq"))
                        nc.tensor.matmul(
                            psq[0],
                            wt[:, kc * 256 + base : kc * 256 + base + 128],
                            xTs[kc][:, h2 * 512 : (h2 + 1) * 512],
                            start=(kc == 0),
                            stop=(kc == KC - 1),
                        )

                    return op

                def mk_cast(dst=dst, h2=h2, psq=psq):
                    def op():
                        nc.vector.tensor_copy(
                            out=dst[:, h2 * 512 : (h2 + 1) * 512], in_=psq[0]
                        )

                    return op

                mms.append([mk_mm(kc) for kc in range(KC)])
                casts.append(mk_cast())
        return (
            mms[0]
            + [casts[0]]
            + mms[1]
            + [casts[1]]
            + mms[2]
            + [casts[2]]
            + mms[3]
            + [casts[3]]
        )

    def emit_qk_block(j, qT, kT, wt):
        for op in qk_ops(j, qT, kT, wt):
            op()

    SKEW = 4

    def emit_attn(j, qT, kT, aT, bg, bg_sched=None):
        # One [128,1024] S tile per nk chunk: head A in cols 0:512, head B in
        # 512:1024 — a single EXP covers both heads. Chunk-granular skew-4
        # software pipeline; ~2 background (qk GEMM) ops drained per unit.
        steps = []
        oab = {}
        # Explicit background-op schedule: 24 qk matmuls + 4 casts. Keep ~2
        # ops on nearly every unit so the PE always has more work per unit
        # than the EXP cadence (a unit with no bg work runs at EXP+semaphore
        # latency — the 2-deep S rotation couples them); leave the O'-drain
        # units 10-11 empty so the DVE drain copies never queue behind a
        # cast; pull cast #3 ahead of the window (3-op units 7-9) and land
        # cast #4 by unit 14, well before the next pair's S matmuls read it.
        BG_SCHED = bg_sched or [2, 2, 2, 2, 2, 2, 2, 3, 3, 3, 0, 0, 2, 2, 1, 1, 0, 0, 0, 0]
        bgi = 0
        for step in range(NQT * NT + SKEW):
            if step < NQT * NT:
                nq, nkc = divmod(step, NT)
                s = ps_big.tile([128, 1024], F32, tag="ps_big", name="sAB")
                for half, kt0 in ((0, 0), (1, 64)):
                    nc.tensor.matmul(
                        s[:, half * 512 : (half + 1) * 512],
                        kT[kt0 : kt0 + 64, nkc * 128 : (nkc + 1) * 128],
                        qT[kt0 : kt0 + 64, nq * 512 : (nq + 1) * 512],
                        tile_position=(kt0, 0),
                    )
                e = expp.tile([128, 1024], BF16, tag="expp", name="eAB")
                nc.scalar.activation(out=e, in_=s, func=EXP, scale=SCALE)
                steps.append((nq, nkc, e))
            if step >= SKEW:
                nq, nkc, e = steps[step - SKEW]
                if nkc == 0:
                    oab[nq] = (
                        ps_o.tile([VW, 512], F32, tag="ps_o", name="oA"),
                        ps_o.tile([VW, 512], F32, tag="ps_o", name="oB"),
                    )
                oA, oB = oab[nq]
                v2v = v2s[nkc].rearrange("p (h e) -> p h e", e=VW)
                nc.tensor.matmul(
                    oA, v2v[:, 2 * j, :], e[:, 0:512], start=(nkc == 0), stop=(nkc == NT - 1)
                )
                nc.tensor.matmul(
                    oB,
                    v2v[:, 2 * j + 1, :],
                    e[:, 512:1024],
                    start=(nkc == 0),
                    stop=(nkc == NT - 1),
                )
                if nkc == NT - 1:
                    for o, half in ((oA, 0), (oB, 1)):
                        # Drain O' to SBUF at once so the PSUM bank frees for
                        # the next nq tile. Remaining normalize work is staged
                        # across later pairs so no DVE op waits on a DMA. The
                        # very last drain's bounce rides the otherwise-idle
                        # gpsimd queue so the two chains don't serialize on
                        # sync's issue rate right before proj.
                        tail = j == PAIRS - 1 and nq == NQT - 1
                        deng = nc.gpsimd if (tail and half == 1) else nc.sync
                        oc = rdp.tile([HD + 1, 512], F32, tag="oc")
                        nc.vector.tensor_copy(out=oc, in_=o[0 : HD + 1, :])
                        scr = dram_scr.tile([1, 512], F32, tag="scr")
                        deng.dma_start(out=scr, in_=oc[HD : HD + 1, :])
                        rs = rdp.tile([128, 4], F32, tag="rs")
                        deng.dma_start(
                            out=rs,
                            in_=bass.AP(
                                tensor=scr.tensor, offset=scr.offset, ap=[[4, 128], [1, 4]]
                            ),
                        )
                        pend1.append((oc, rs, aT, half, nq, deng))
            if bg:
                take = min(BG_SCHED[step], len(bg) - bgi)
                for _ in range(take):
                    bg[bgi]()
                    bgi += 1
            if j == PAIRS - 1 and step == NQT * NT - 1:
                # Last pair, before the drain tail: flush nq=0's normalize
                # chain now (its bounce resolved ~4 units ago). aT5's first
                # 512 columns are then ready the moment the pair ends, so
                # proj tiles nt=0..3 never stall on the kc=5 contraction;
                # nt=4..7 run ~12us later, by when nq=1's chain has landed.
                emit_norm1()
                emit_norm2()
        while bgi < len(bg):
            bg[bgi]()
            bgi += 1

    def emit_norm1():
        for oc, rs, aT, half, nq, deng in pend1:
            rs2 = rdp.tile([128, 4], F32, tag="rs2")
            nc.vector.reciprocal(out=rs2, in_=rs)
            scr2 = dram_scr.tile([1, 512], F32, tag="scr2")
            deng.dma_start(
                out=bass.AP(tensor=scr2.tensor, offset=scr2.offset, ap=[[4, 128], [1, 4]]),
                in_=rs2,
            )
            rb = rdp.tile([64, 512], F32, tag="rb")
            deng.dma_start(
                out=rb,
                in_=bass.AP(
                    tensor=scr2.tensor, offset=scr2.offset, ap=[[0, 64]] + list(scr2.ap[1:])
                ),
            )
            pend2.append((oc, rb, aT, half, nq))
        pend1.clear()

    def emit_norm2():
        for oc, rb, aT, half, nq in pend2:
            nc.vector.tensor_mul(
                out=aT[half * 64 : half * 64 + 64, nq * 512 : (nq + 1) * 512],
                in0=oc[0:HD, :],
                in1=rb,
            )
        pend2.clear()

    proj_nt0 = {}

    def tail_ops():
        # pair 5 has no next-pair qk GEMM to interleave, so it runs at the
        # EXP-coupled cadence with the PE ~25% idle. Fill it: flush pair 4's
        # normalize multiplies (their bounces resolved by unit 2), then run
        # proj tile nt=0's first 5 contraction chunks on the free ps_q banks
        # — phase E only finalizes kc=5 once aT5 exists.
        ops = [emit_norm2]

        def mk(kc):
            def op():
                if kc == 0:
                    proj_nt0["a"] = ps_q.tile([128, 512], F32, tag="ps_q", name="psy512")
                    proj_nt0["b"] = ps_q.tile([128, 256], F32, tag="ps_q", name="psy256")
                lhsT = aTs[kc][:, 0:128]
                nc.tensor.matmul(
                    proj_nt0["a"],
                    lhsT,
                    wps[kc][:, 0:512],
                    start=(kc == 0),
                    stop=False,
                    skip_group_check=True,
                )
                nc.tensor.matmul(
                    proj_nt0["b"],
                    lhsT,
                    wps[kc][:, 512:768],
                    start=(kc == 0),
                    stop=False,
                    skip_group_check=True,
                )

            return op

        ops += [mk(kc) for kc in range(KC - 1)]
        sched = [0, 0, 1, 0, 1, 1, 1, 1, 1, 0, 0, 0, 0, 0, 0, 0, 0, 0, 0, 0]
        return ops, sched

    aTs = []
    qkts = []
    pend1 = []
    pend2 = []
    wts = {}
    for j in range(PAIRS):
        qkts.append(
            (
                persist.tile([128, N], BF16, tag=f"qT{j}", name=f"qT{j}"),
                persist.tile([128, N], BF16, tag=f"kT{j}", name=f"kT{j}"),
            )
        )
    emit_warmup()
    wts[0] = load_wqk(0, nc.sync)
    wts[1] = load_wqk(1, nc.scalar)
    emit_v()
    emit_qk_block(0, *qkts[0], wts[0])
    for j in range(PAIRS):
        if j + 2 < PAIRS:
            wts[j + 2] = load_wqk(j + 2, nc.sync)
        aT = persist.tile([128, N], BF16, tag=f"aT{j}", name=f"aT{j}")
        aTs.append(aT)
        if j + 1 < PAIRS:
            bg, sched = qk_ops(j + 1, *qkts[j + 1], wts[j + 1]), None
        else:
            bg, sched = tail_ops()
        emit_norm2()  # pair j-2 multiplies (bounce-back resolved long ago)
        emit_norm1()  # pair j-1 reciprocal + bounce-back (spread resolved)
        emit_attn(j, *qkts[j], aT, bg, sched)
        if j == 3:
            # Late enough to stay off the qk-weight prefetch window, early
            # enough (~60us before proj) to never gate it.
            wps = emit_wp_loads()
    emit_norm1()
    emit_norm2()

    # ---- phase E: proj + bias ----
    # Attention is done, so ps_q and ps_o are free: alternating psy between
    # (ps_q+ps_o) and ps_big gives an effective rotation depth of 4.
    eng2 = [nc.sync, nc.scalar]
    for nt in range(NT):
        if nt == 0:
            # kc 0..4 accumulated during pair 5's attention (tail_ops); only
            # the aT5 contraction chunk remains.
            ps512, ps256 = proj_nt0["a"], proj_nt0["b"]
            lhsT = aTs[KC - 1][:, 0:128]
            nc.tensor.matmul(
                ps512, lhsT, wps[KC - 1][:, 0:512], start=False, stop=True,
                skip_group_check=True,
            )
            nc.tensor.matmul(
                ps256, lhsT, wps[KC - 1][:, 512:768], start=False, stop=True,
                skip_group_check=True,
            )
        else:
            if nt % 2 == 0:
                ps512 = ps_q.tile([128, 512], F32, tag="ps_q", name="psy512")
                ps256 = ps_o.tile([128, 256], F32, tag="ps_o", name="psy256")
            else:
                psy = ps_big.tile([128, 1024], F32, tag="ps_big", name="psy")
                ps512, ps256 = psy[:, 0:512], psy[:, 512:768]
            for kc in range(KC):
                lhsT = aTs[kc][:, nt * 128 : (nt + 1) * 128]
                nc.tensor.matmul(
                    ps512, lhsT, wps[kc][:, 0:512], start=(kc == 0), stop=(kc == KC - 1)
                )
                nc.tensor.matmul(
                    ps256, lhsT, wps[kc][:, 512:768], start=(kc == 0), stop=(kc == KC - 1)
                )
        yb = work.tile([128, C], F32, tag="yb")
        nc.vector.tensor_add(out=yb[:, 0:512], in0=ps512, in1=bpb[:, 0:512])
        nc.vector.tensor_add(out=yb[:, 512:768], in0=ps256, in1=bpb[:, 512:768])
        for h in range(2):
            eng2[h].dma_start(
                out=y[nt * 128 : (nt + 1) * 128, h * 384 : (h + 1) * 384],
                in_=yb[:, h * 384 : (h + 1) * 384],
            )


def build():
    from contextlib import ExitStack

    nc = bacc.Bacc("TRN2", target_bir_lowering=False, debug=False)
    xT = nc.dram_tensor("xT", [C, N], BF16, kind="ExternalInput").ap()
    wqkp = nc.dram_tensor("wqkp", [PAIRS * C, 256], BF16, kind="ExternalInput").ap()
    wv = nc.dram_tensor("wv", [C, C], BF16, kind="ExternalInput").ap()
    wpT = nc.dram_tensor("wpT", [C, C], BF16, kind="ExternalInput").ap()
    bproj = nc.dram_tensor("bproj", [C], F32, kind="ExternalInput").ap()
    y = nc.dram_tensor("y", [N, C], F32, kind="ExternalOutput").ap()
    with tile.TileContext(nc) as tc:
        with ExitStack() as ctx:
            _emit(tc, nc, xT, wqkp, wv, wpT, bproj, y, ctx)
    nc.compile()
    return nc


_NC_CACHE = {}


def make_in_maps(x, w_qkv, w_proj, b_proj):
    import ml_dtypes

    bf16 = ml_dtypes.bfloat16
    wqkvT = np.asarray(w_qkv).T  # [C, 3C]; cols 0:C=q, C:2C=k, 2C:3C=v
    blocks = [
        np.concatenate(
            [wqkvT[:, j * 128 : (j + 1) * 128], wqkvT[:, C + j * 128 : C + (j + 1) * 128]],
            axis=1,
        )
        for j in range(PAIRS)
    ]
    wqkp = np.ascontiguousarray(np.concatenate(blocks, axis=0)).astype(bf16)
    wv = np.ascontiguousarray(wqkvT[:, 2 * C : 3 * C]).astype(bf16)
    wpT = np.ascontiguousarray(np.asarray(w_proj).T).astype(bf16)
    b_proj = np.asarray(b_proj, dtype=np.float32)
    return [
        {
            "xT": np.ascontiguousarray(np.asarray(x[b]).T).astype(bf16),
            "wqkp": wqkp,
            "wv": wv,
            "wpT": wpT,
            "bproj": b_proj,
        }
        for b in range(NCORES)
    ]


def kernel(x, w_qkv, w_proj, b_proj, _trace=False, _tmpdir=None):
    if "nc" not in _NC_CACHE:
        _NC_CACHE["nc"] = build()
    nc = _NC_CACHE["nc"]
    in_maps = make_in_maps(x, w_qkv, w_proj, b_proj)
    kwargs = {}
    if _trace:
        kwargs = {"trace": True, "tmpdir": _tmpdir}
    res = run_bass_kernel_spmd(nc, in_maps, core_ids=list(range(NCORES)), **kwargs)
    out = np.stack([res.results[i]["y"] for i in range(NCORES)], axis=0)
    if _trace:
        _NC_CACHE["last_result"] = res
    return out


if __name__ == "__main__":
    rng = np.random.default_rng(0)
    x = rng.standard_normal((B, N, C), dtype=np.float32)
    w_qkv = (rng.standard_normal((3 * C, C), dtype=np.float32) * C**-0.5).astype(np.float32)
    w_proj = (rng.standard_normal((C, C), dtype=np.float32) * C**-0.5).astype(np.float32)
    b_proj = np.zeros(C, dtype=np.float32)
    out = kernel(x, w_qkv, w_proj, b_proj)
    print("out", out.shape, out.dtype, float(np.abs(out).mean()))
